# revision 1
# baseline (speedup 1.0000x reference)
"""GroupSparseAE (FISTA group-lasso encoder + linear decoder) on 8 trn2 cores.

Data-parallel over batch: each core gets B/8 = 64 rows, W replicated.
Per channel c (3 total, processed sequentially so W[c]/W[c]^T fit in SBUF):
  y2   = TAU * (W @ x^T)                   [D, b] transposed layout
  FISTA iterate k = 1..30 with x in transposed [D, b] layout:
    u^T    = W^T-contract:  uT[n,b]   = sum_d W[d,n] xT[d,b]
    grad^T = gT[e,b]        = sum_n WT[n,e] uT[n,b]
    v      = xT_tmp + y2 - TAU*gT
    group soft-threshold (groups of 8 along d = partition dim):
       gs = Bmat^T @ v^2  (Bmat block-diag ones -> broadcast group sumsq)
       xnew = relu(v) * relu(1 - c/sqrt(gs))
    momentum: xtmp = xnew + m_k (xnew - xold)
  decode: out^T[n,b] = sum_d W[d,n] z[d,b]
All matmuls: stationary [128,128] weight tile, moving [128,64] activation
slice, fp32 accumulate in PSUM.
"""

import sys

sys.path.insert(0, "/opt/trn_rl_repo")

import numpy as np

B, C, N = 512, 3, 1024
G, S = 256, 8
D = G * S  # 2048
NUM_LAYERS = 30
TAU, LAM = 0.1, 0.1
CTH = LAM * TAU  # group threshold constant

N_CORES = 8
BL = B // N_CORES  # 64 rows per core
NT = D // 128  # 16 d-tiles
NS = N // 128  # 8 n-tiles
FD = NT * BL  # 1024 flat free dim of [D, b] state
CHUNK = 256  # elementwise chunk (4 d-tiles)
NCH = FD // CHUNK


def _mom_coeffs(num_layers):
    # fp32 t-sequence to match the reference's on-device arithmetic
    one, four, two = np.float32(1.0), np.float32(4.0), np.float32(2.0)
    t = np.float32(1.0)
    ms = []
    for _ in range(num_layers):
        t_new = (one + np.sqrt(one + four * t * t)) / two
        ms.append(float((t - one) / t_new))
        t = t_new
    return ms


def _bmat_np():
    p = np.arange(128)
    return (p[:, None] // S == p[None, :] // S).astype(np.float32)


def build(num_layers=NUM_LAYERS):
    import concourse.bacc as bacc
    from concourse import mybir
    from concourse.tile import TileContext

    fp32 = mybir.dt.float32
    AF = mybir.ActivationFunctionType
    OP = mybir.AluOpType

    nc = bacc.Bacc("TRN2", target_bir_lowering=False, debug=False,
                   num_devices=N_CORES)
    xt = nc.dram_tensor("xt", [C, N, BL], fp32, kind="ExternalInput")
    w = nc.dram_tensor("w", [C, D, N], fp32, kind="ExternalInput")
    wt = nc.dram_tensor("wt", [C, N, D], fp32, kind="ExternalInput")
    bm = nc.dram_tensor("bm", [128, 128], fp32, kind="ExternalInput")
    ot = nc.dram_tensor("ot", [C, N, BL], fp32, kind="ExternalOutput")

    ms = _mom_coeffs(num_layers)

    with TileContext(nc) as tc:
        with (
            tc.tile_pool(name="wp", bufs=1) as wp,
            tc.tile_pool(name="st", bufs=1) as st,
            tc.tile_pool(name="scr", bufs=4) as scr,
            tc.tile_pool(name="ps_u", bufs=2, space="PSUM") as ps_u,
            tc.tile_pool(name="ps_g", bufs=3, space="PSUM") as ps_g,
            tc.tile_pool(name="ps_s", bufs=2, space="PSUM") as ps_s,
        ):
            bmat = wp.tile([128, 128], fp32, tag="bmat")
            nc.sync.dma_start(out=bmat, in_=bm[:, :])

            for c in range(C):
                wsb = wp.tile([128, NT, N], fp32, tag="wsb")
                nc.sync.dma_start(
                    out=wsb, in_=w[c].rearrange("(t p) n -> p t n", p=128))
                wtsb = wp.tile([128, NS, D], fp32, tag="wtsb")
                nc.sync.dma_start(
                    out=wtsb, in_=wt[c].rearrange("(s p) e -> p s e", p=128))
                xts = wp.tile([128, NS, BL], fp32, tag="xts")
                nc.sync.dma_start(
                    out=xts, in_=xt[c].rearrange("(s p) b -> p s b", p=128))

                # persistent per-channel state
                y2 = st.tile([128, FD], fp32, tag="y2")
                xb0 = st.tile([128, FD], fp32, tag="xb0")
                xb1 = st.tile([128, FD], fp32, tag="xb1")
                xbuf = [xb0, xb1]
                uT = st.tile([128, NS * BL], fp32, tag="uT")
                # chunked tiles for cross-iteration pipelining
                xtmp = [st.tile([128, CHUNK], fp32, tag=f"xtmp{j}",
                                name=f"xtmp{j}") for j in range(NCH)]
                pre = [st.tile([128, CHUNK], fp32, tag=f"pre{j}",
                               name=f"pre{j}") for j in range(NCH)]

                nc.vector.memset(xb0, 0.0)

                # ---- precomp: y2 = TAU * W @ x^T  in [D, b] layout ----
                for t in range(NT):
                    py = ps_g.tile([128, BL], fp32, tag="pg")
                    for s in range(NS):
                        nc.tensor.matmul(
                            py, wtsb[:, s, t * 128:(t + 1) * 128],
                            xts[:, s, :], start=(s == 0), stop=(s == NS - 1))
                    nc.scalar.mul(y2[:, t * BL:(t + 1) * BL], py, TAU)

                def act_block(vch, k):
                    """vch(j) -> [128, CHUNK] AP of the pre-activation v.
                    Writes xnew (xbuf[k % 2]); unless last iter, also xtmp/pre.
                    """
                    xnew, xold = xbuf[k % 2], xbuf[(k - 1) % 2]
                    m = ms[k - 1]
                    last = k == num_layers
                    for j in range(NCH):
                        sl = slice(j * CHUNK, (j + 1) * CHUNK)
                        vj = vch(j)
                        v2 = scr.tile([128, CHUNK], fp32, tag="v2")
                        nc.scalar.square(v2, vj)
                        gs = ps_s.tile([128, CHUNK], fp32, tag="gs")
                        nc.tensor.matmul(gs, bmat, v2, start=True, stop=True)
                        nrm = scr.tile([128, CHUNK], fp32, tag="nrm")
                        nc.scalar.sqrt(nrm, gs)
                        invn = scr.tile([128, CHUNK], fp32, tag="invn")
                        nc.vector.reciprocal(invn, nrm)
                        scl = scr.tile([128, CHUNK], fp32, tag="scl")
                        # relu(1 - CTH / nrm)
                        nc.scalar.activation(scl, invn, AF.Relu,
                                             bias=1.0, scale=-CTH)
                        # xnew = max(v, 0) * scl
                        nc.vector.scalar_tensor_tensor(
                            xnew[:, sl], vj, 0.0, scl,
                            op0=OP.max, op1=OP.mult)
                        if not last:
                            dd = scr.tile([128, CHUNK], fp32, tag="dd")
                            nc.vector.tensor_sub(dd, xnew[:, sl], xold[:, sl])
                            nc.vector.scalar_tensor_tensor(
                                xtmp[j], dd, m, xnew[:, sl],
                                op0=OP.mult, op1=OP.add)
                            nc.vector.tensor_add(pre[j], xtmp[j], y2[:, sl])

                # ---- iteration 1: x_tmp = 0 -> v = y2 ----
                act_block(lambda j: y2[:, j * CHUNK:(j + 1) * CHUNK], 1)

                # ---- iterations 2..num_layers ----
                for k in range(2, num_layers + 1):
                    # u-phase: uT[n,b] = sum_d W[d,n] xtmp[d,b]
                    for s in range(NS):
                        pu = ps_u.tile([128, BL], fp32, tag="pu")
                        for t in range(NT):
                            nc.tensor.matmul(
                                pu, wsb[:, t, s * 128:(s + 1) * 128],
                                xtmp[t // 4][:, (t % 4) * BL:(t % 4 + 1) * BL],
                                start=(t == 0), stop=(t == NT - 1))
                        nc.scalar.copy(uT[:, s * BL:(s + 1) * BL], pu)
                    # grad-phase + v-combine
                    vt = [scr.tile([128, CHUNK], fp32, tag=f"v{j}", name=f"v{j}")
                          for j in range(NCH)]
                    for t in range(NT):
                        pg = ps_g.tile([128, BL], fp32, tag="pg")
                        for s in range(NS):
                            nc.tensor.matmul(
                                pg, wtsb[:, s, t * 128:(t + 1) * 128],
                                uT[:, s * BL:(s + 1) * BL],
                                start=(s == 0), stop=(s == NS - 1))
                        # v = pre - TAU * grad
                        nc.vector.scalar_tensor_tensor(
                            vt[t // 4][:, (t % 4) * BL:(t % 4 + 1) * BL],
                            pg, -TAU, pre[t // 4][:, (t % 4) * BL:(t % 4 + 1) * BL],
                            op0=OP.mult, op1=OP.add)
                    act_block(lambda j: vt[j][:, :], k)

                # ---- decode: out^T[n,b] = sum_d W[d,n] z[d,b] ----
                z = xbuf[num_layers % 2]
                otsb = st.tile([128, NS, BL], fp32, tag="otsb")
                for s in range(NS):
                    pd = ps_u.tile([128, BL], fp32, tag="pu")
                    for t in range(NT):
                        nc.tensor.matmul(
                            pd, wsb[:, t, s * 128:(s + 1) * 128],
                            z[:, t * BL:(t + 1) * BL],
                            start=(t == 0), stop=(t == NT - 1))
                    nc.scalar.copy(otsb[:, s, :], pd)
                nc.sync.dma_start(
                    out=ot[c].rearrange("(s p) b -> p s b", p=128), in_=otsb)

    nc.compile()
    return nc


_CACHED = {}


def _get_nc(num_layers=NUM_LAYERS):
    if num_layers not in _CACHED:
        _CACHED[num_layers] = build(num_layers)
    return _CACHED[num_layers]


def make_in_maps(x, w):
    """x [B,C,N] fp32, w [C,D,N] fp32 -> list of 8 per-core input dicts."""
    x = np.asarray(x, dtype=np.float32)
    w = np.ascontiguousarray(np.asarray(w, dtype=np.float32))
    wt = np.ascontiguousarray(w.transpose(0, 2, 1))
    bm = _bmat_np()
    maps = []
    for i in range(N_CORES):
        xs = x[i * BL:(i + 1) * BL]  # [BL, C, N]
        xts = np.ascontiguousarray(xs.transpose(1, 2, 0))  # [C, N, BL]
        maps.append({"xt": xts, "w": w, "wt": wt, "bm": bm})
    return maps


def assemble_out(results):
    outs = []
    for i in range(N_CORES):
        o = results[i]["ot"]  # [C, N, BL]
        outs.append(np.ascontiguousarray(o.transpose(2, 0, 1)))  # [BL, C, N]
    return np.concatenate(outs, axis=0).astype(np.float32)


def kernel(x, W):
    from concourse.bass_utils import run_bass_kernel_spmd

    nc = _get_nc()
    res = run_bass_kernel_spmd(nc, make_in_maps(x, W), list(range(N_CORES)))
    return assemble_out(res.results)


if __name__ == "__main__":
    xs = np.random.randn(B, C, N).astype(np.float32)
    ws = np.random.randn(C, D, N).astype(np.float32)
    ws /= np.linalg.norm(ws, axis=-1, keepdims=True)
    out = kernel(xs, ws)
    print("out", out.shape, out.dtype, float(np.abs(out).mean()))



# revision 4
# speedup vs baseline: 6.6964x; 6.6964x over previous
"""GroupSparseAE (FISTA group-lasso encoder + linear decoder) on 8 trn2 cores.

Sharding: one channel x 256 batch columns per core (channel c = B/2 split).
Cores 0-5 cover (c, half) = (0,0),(0,1),(1,0),(1,1),(2,0),(2,1); cores 6,7
duplicate channel 2 (SPMD needs a uniform program; 3 channels don't divide 8)
and their outputs are discarded.

All matmuls run in bf16 (fp32 PSUM accumulation): 1 cycle/row on the PE vs 4
for fp32, and the 256-wide moving dim amortizes each 128x128 stationary load.
State layout is transposed [D, b] so the contraction dim (d or n) is always on
partitions and no transposes are needed anywhere:
  y2     = TAU * (W @ x^T)                  [D, b]
  FISTA iterate k = 1..30:
    uT[n,b]   = sum_d W[d,n] xtmp[d,b]      (moving = xtmp, bf16)
    gT[e,b]   = sum_n WT[n,e] uT[n,b]       (moving = uT, bf16)
    v         = pre - TAU*gT                (pre = xtmp + y2, kept fresh)
    group soft-threshold (groups of 8 along d = partition dim):
      gs   = Bmat^T @ v^2   (block-diag ones matmul -> broadcast group sumsq)
      xnew = relu(v) * relu(1 - c/sqrt(gs))
    momentum: xtmp = xnew + m_k (xnew - xold)   (written bf16 for the matmul)
  decode: out^T[n,b] = sum_d W[d,n] z[d,b]
"""

import sys

sys.path.insert(0, "/opt/trn_rl_repo")

import numpy as np
import ml_dtypes

B, C, N = 512, 3, 1024
G, S = 256, 8
D = G * S  # 2048
NUM_LAYERS = 30
TAU, LAM = 0.1, 0.1
CTH = LAM * TAU  # group threshold constant
EPS = 1e-30  # guard for 1/sqrt(0) in the approx reciprocal

N_CORES = 8
BL = 256  # batch columns per core (one channel, half the batch)
NT = D // 128  # 16 d-tiles
NS = N // 128  # 8 n-tiles
FD = NT * BL  # 4096 flat free dim of [D, b] state
CHUNK = 2 * BL  # elementwise chunk = 2 d-tiles
NCH = FD // CHUNK  # 8

CORE_CH = [0, 0, 1, 1, 2, 2, 2, 2]
CORE_HALF = [0, 1, 0, 1, 0, 1, 0, 1]
REAL_CORES = list(range(6))  # outputs of cores 6,7 are duplicates


def _mom_coeffs(num_layers):
    # fp32 t-sequence to match the reference's on-device arithmetic
    one, four, two = np.float32(1.0), np.float32(4.0), np.float32(2.0)
    t = np.float32(1.0)
    ms = []
    for _ in range(num_layers):
        t_new = (one + np.sqrt(one + four * t * t)) / two
        ms.append(float((t - one) / t_new))
        t = t_new
    return ms


def _bmat_np():
    p = np.arange(128)
    return (p[:, None] // S == p[None, :] // S).astype(ml_dtypes.bfloat16)


def build(num_layers=NUM_LAYERS):
    import concourse.bacc as bacc
    from concourse import mybir
    from concourse.tile import TileContext

    fp32 = mybir.dt.float32
    bf16 = mybir.dt.bfloat16
    AF = mybir.ActivationFunctionType
    OP = mybir.AluOpType

    nc = bacc.Bacc("TRN2", target_bir_lowering=False, debug=False,
                   num_devices=N_CORES)
    xt = nc.dram_tensor("xt", [N, BL], bf16, kind="ExternalInput")
    w = nc.dram_tensor("w", [D, N], bf16, kind="ExternalInput")
    wt = nc.dram_tensor("wt", [N, D], bf16, kind="ExternalInput")
    bm = nc.dram_tensor("bm", [128, 128], bf16, kind="ExternalInput")
    ot = nc.dram_tensor("ot", [N, BL], fp32, kind="ExternalOutput")

    ms = _mom_coeffs(num_layers)

    with TileContext(nc) as tc:
        with (
            tc.tile_pool(name="wp", bufs=1) as wp,
            tc.tile_pool(name="st", bufs=1) as st,
            tc.tile_pool(name="scr", bufs=3) as scr,
            tc.tile_pool(name="ps_u", bufs=2, space="PSUM") as ps_u,
            tc.tile_pool(name="ps_g", bufs=2, space="PSUM") as ps_g,
            tc.tile_pool(name="ps_s", bufs=2, space="PSUM") as ps_s,
        ):
            bmat = wp.tile([128, 128], bf16, tag="bmat")
            nc.sync.dma_start(out=bmat, in_=bm[:, :])
            eps_t = wp.tile([128, 1], fp32, tag="eps")
            nc.vector.memset(eps_t, EPS)
            wtsb = wp.tile([128, NS, D], bf16, tag="wtsb")
            nc.sync.dma_start(
                out=wtsb, in_=wt.rearrange("(s p) e -> p s e", p=128))
            xts = wp.tile([128, NS, BL], bf16, tag="xts")
            nc.sync.dma_start(
                out=xts, in_=xt.rearrange("(s p) b -> p s b", p=128))
            wsb = wp.tile([128, NT, N], bf16, tag="wsb")
            nc.sync.dma_start(
                out=wsb, in_=w.rearrange("(t p) n -> p t n", p=128))

            # persistent state
            y2 = st.tile([128, FD], fp32, tag="y2")
            xb0 = st.tile([128, FD], fp32, tag="xb0")
            xb1 = st.tile([128, FD], fp32, tag="xb1")
            xbuf = [xb0, xb1]
            uTb = st.tile([128, NS * BL], bf16, tag="uTb")
            # chunked for cross-iteration pipelining (u-phase starts on
            # chunk j as soon as act chunk j lands)
            xtmpb = [st.tile([128, CHUNK], bf16, tag=f"xtmp{j}",
                             name=f"xtmp{j}") for j in range(NCH)]
            pre = [st.tile([128, CHUNK], fp32, tag=f"pre{j}",
                           name=f"pre{j}") for j in range(NCH)]
            vt = [st.tile([128, CHUNK], fp32, tag=f"v{j}",
                          name=f"v{j}") for j in range(NCH)]

            nc.vector.memset(xb0, 0.0)

            # ---- precomp: y2 = TAU * W @ x^T  in [D, b] layout ----
            for t in range(NT):
                py = ps_g.tile([128, BL], fp32, tag="pg")
                for s in range(NS):
                    nc.tensor.matmul(
                        py, wtsb[:, s, t * 128:(t + 1) * 128],
                        xts[:, s, :], start=(s == 0), stop=(s == NS - 1))
                nc.scalar.mul(y2[:, t * BL:(t + 1) * BL], py, TAU)

            def act_block(vch, k):
                """vch(j) -> [128, CHUNK] AP of the pre-activation v.
                Writes xnew + (xtmpb, pre) for the next iteration; on the
                last iteration writes z (bf16) into the xtmpb tiles instead.
                """
                xnew, xold = xbuf[k % 2], xbuf[(k - 1) % 2]
                m = ms[k - 1]
                last = k == num_layers
                for j in range(NCH):
                    sl = slice(j * CHUNK, (j + 1) * CHUNK)
                    vj = vch(j)
                    v2b = scr.tile([128, CHUNK], bf16, tag="v2b")
                    nc.scalar.square(v2b, vj)
                    gs = ps_s.tile([128, CHUNK], fp32, tag="gs")
                    nc.tensor.matmul(gs, bmat, v2b, start=True, stop=True)
                    nrm = scr.tile([128, CHUNK], fp32, tag="nrm")
                    nc.scalar.activation(nrm, gs, AF.Sqrt, bias=eps_t[:, :])
                    invn = scr.tile([128, CHUNK], fp32, tag="invn")
                    nc.vector.reciprocal_approx_fast(invn, nrm)
                    scl = scr.tile([128, CHUNK], fp32, tag="scl")
                    # relu(1 - CTH / nrm)
                    nc.scalar.activation(scl, invn, AF.Relu,
                                         bias=1.0, scale=-CTH)
                    if last:
                        # z (bf16) straight into the xtmp tiles for decode
                        nc.vector.scalar_tensor_tensor(
                            xtmpb[j], vj, 0.0, scl, op0=OP.max, op1=OP.mult)
                        continue
                    # xnew = max(v, 0) * scl
                    nc.vector.scalar_tensor_tensor(
                        xnew[:, sl], vj, 0.0, scl, op0=OP.max, op1=OP.mult)
                    dd = scr.tile([128, CHUNK], fp32, tag="dd")
                    nc.vector.tensor_sub(dd, xnew[:, sl], xold[:, sl])
                    nc.vector.scalar_tensor_tensor(
                        xtmpb[j], dd, m, xnew[:, sl],
                        op0=OP.mult, op1=OP.add)
                    nc.vector.tensor_add(pre[j], xtmpb[j], y2[:, sl])

            # ---- iteration 1: x_tmp = 0 -> v = y2 ----
            act_block(lambda j: y2[:, j * CHUNK:(j + 1) * CHUNK], 1)

            # ---- iterations 2..num_layers ----
            for k in range(2, num_layers + 1):
                # u-phase: uT[n,b] = sum_d W[d,n] xtmp[d,b]
                for s in range(NS):
                    pu = ps_u.tile([128, BL], fp32, tag="pu")
                    for t in range(NT):
                        nc.tensor.matmul(
                            pu, wsb[:, t, s * 128:(s + 1) * 128],
                            xtmpb[t // 2][:, (t % 2) * BL:(t % 2 + 1) * BL],
                            start=(t == 0), stop=(t == NT - 1))
                    nc.scalar.copy(uTb[:, s * BL:(s + 1) * BL], pu)
                # grad-phase + v-combine
                for t in range(NT):
                    pg = ps_g.tile([128, BL], fp32, tag="pg")
                    for s in range(NS):
                        nc.tensor.matmul(
                            pg, wtsb[:, s, t * 128:(t + 1) * 128],
                            uTb[:, s * BL:(s + 1) * BL],
                            start=(s == 0), stop=(s == NS - 1))
                    # v = pre - TAU * grad
                    nc.vector.scalar_tensor_tensor(
                        vt[t // 2][:, (t % 2) * BL:(t % 2 + 1) * BL],
                        pg, -TAU,
                        pre[t // 2][:, (t % 2) * BL:(t % 2 + 1) * BL],
                        op0=OP.mult, op1=OP.add)
                act_block(lambda j: vt[j][:, :], k)

            # ---- decode: out^T[n,b] = sum_d W[d,n] z[d,b] ----
            otsb = st.tile([128, NS, BL], fp32, tag="otsb")
            for s in range(NS):
                pd = ps_u.tile([128, BL], fp32, tag="pu")
                for t in range(NT):
                    nc.tensor.matmul(
                        pd, wsb[:, t, s * 128:(s + 1) * 128],
                        xtmpb[t // 2][:, (t % 2) * BL:(t % 2 + 1) * BL],
                        start=(t == 0), stop=(t == NT - 1))
                nc.scalar.copy(otsb[:, s, :], pd)
            nc.sync.dma_start(
                out=ot.rearrange("(s p) b -> p s b", p=128), in_=otsb)

    nc.compile()
    return nc


_CACHED = {}


def _get_nc(num_layers=NUM_LAYERS):
    if num_layers not in _CACHED:
        _CACHED[num_layers] = build(num_layers)
    return _CACHED[num_layers]


def make_in_maps(x, w):
    """x [B,C,N] fp32, w [C,D,N] fp32 -> list of 8 per-core input dicts."""
    x = np.asarray(x, dtype=np.float32)
    w = np.asarray(w, dtype=np.float32)
    bm = _bmat_np()
    wc = [np.ascontiguousarray(w[c]).astype(ml_dtypes.bfloat16)
          for c in range(C)]
    wtc = [np.ascontiguousarray(w[c].T).astype(ml_dtypes.bfloat16)
           for c in range(C)]
    maps = []
    for i in range(N_CORES):
        c, h = CORE_CH[i], CORE_HALF[i]
        xs = x[h * BL:(h + 1) * BL, c, :]  # [BL, N]
        xts = np.ascontiguousarray(xs.T).astype(ml_dtypes.bfloat16)  # [N, BL]
        maps.append({"xt": xts, "w": wc[c], "wt": wtc[c], "bm": bm})
    return maps


def assemble_out(results):
    out = np.empty((B, C, N), dtype=np.float32)
    for i in REAL_CORES:
        c, h = CORE_CH[i], CORE_HALF[i]
        o = np.asarray(results[i]["ot"], dtype=np.float32)  # [N, BL]
        out[h * BL:(h + 1) * BL, c, :] = o.T
    return out


def kernel(x, W):
    from concourse.bass_utils import run_bass_kernel_spmd

    nc = _get_nc()
    res = run_bass_kernel_spmd(nc, make_in_maps(x, W), list(range(N_CORES)))
    return assemble_out(res.results)


if __name__ == "__main__":
    xs = np.random.randn(B, C, N).astype(np.float32)
    ws = np.random.randn(C, D, N).astype(np.float32)
    ws /= np.linalg.norm(ws, axis=-1, keepdims=True)
    out = kernel(xs, ws)
    print("out", out.shape, out.dtype, float(np.abs(out).mean()))


# revision 10
# speedup vs baseline: 7.1010x; 1.0604x over previous
"""GroupSparseAE (FISTA group-lasso encoder + linear decoder) on 8 trn2 cores.

Sharding: one channel x 256 batch columns per core (channel c = B/2 split).
Cores 0-5 cover (c, half) = (0,0),(0,1),(1,0),(1,1),(2,0),(2,1); cores 6,7
duplicate channel 2 (SPMD needs a uniform program; 3 channels don't divide 8)
and their outputs are discarded.

All matmuls run in bf16 (fp32 PSUM accumulation): 1 cycle/row on the PE vs 4
for fp32, and the 256-wide moving dim amortizes each 128x128 stationary load.
State layout is transposed [D, b] so the contraction dim (d or n) is always on
partitions and no transposes are needed anywhere.

Momentum is reformulated to keep the linear term of v in fp32 (bf16 there
costs 4x in final accuracy) at no extra elementwise cost:
  xs   = xnew - m/(1+m) * xold          (fp32; bf16 copy feeds the matmul)
  xtmp = (1+m) * xs                     (the (1+m) folds into the uT copy)
  pre  = (1+m) * xs + y2                (fp32)
  v    = pre - TAU * (W W^T xtmp)       via uT = (1+m) * (W^T xs)

Per iteration:
  u-phase  (t-outer, 8 concurrent PSUM groups packed 2/bank): uT = W^T xtmp
  grad     gT[e,b] = sum_n WT[n,e] uT[n,b];  v = pre - TAU*gT
  act      group soft-threshold via Bmat matmul of v^2 (groups of 8 = along
           partitions), xnew = relu(v)*relu(1-c/sqrt(gs)); next xs/pre
Elementwise work is spread over ACT (square/sqrt/scale), DVE (reciprocal,
xs, bf16 copy, v-combine) and GpSimd (xnew, pre) so the PE stays the
bottleneck.
"""

import sys

sys.path.insert(0, "/opt/trn_rl_repo")

import numpy as np
import ml_dtypes

B, C, N = 512, 3, 1024
G, S = 256, 8
D = G * S  # 2048
NUM_LAYERS = 30
TAU, LAM = 0.1, 0.1
CTH = LAM * TAU  # group threshold constant
EPS = 1e-30  # guard for 1/sqrt(0) in the approx reciprocal

N_CORES = 8
BL = 256  # batch columns per core (one channel, half the batch)
NT = D // 128  # 16 d-tiles
NS = N // 128  # 8 n-tiles
FD = NT * BL  # 4096 flat free dim of [D, b] state
CHUNK = 2 * BL  # elementwise chunk = 2 d-tiles
NCH = FD // CHUNK  # 8

CORE_CH = [0, 0, 1, 1, 2, 2, 2, 2]
CORE_HALF = [0, 1, 0, 1, 0, 1, 0, 1]
REAL_CORES = list(range(6))  # outputs of cores 6,7 are duplicates


def _mom_coeffs(num_layers):
    # fp32 t-sequence to match the reference's on-device arithmetic
    one, four, two = np.float32(1.0), np.float32(4.0), np.float32(2.0)
    t = np.float32(1.0)
    ms = []
    for _ in range(num_layers):
        t_new = (one + np.sqrt(one + four * t * t)) / two
        ms.append(float((t - one) / t_new))
        t = t_new
    return ms


def _bmat_np():
    p = np.arange(128)
    return (p[:, None] // S == p[None, :] // S).astype(ml_dtypes.bfloat16)


def build(num_layers=NUM_LAYERS):
    import concourse.bacc as bacc
    from concourse import mybir
    from concourse.tile import TileContext

    fp32 = mybir.dt.float32
    bf16 = mybir.dt.bfloat16
    AF = mybir.ActivationFunctionType
    OP = mybir.AluOpType

    nc = bacc.Bacc("TRN2", target_bir_lowering=False, debug=False,
                   num_devices=N_CORES)
    xt = nc.dram_tensor("xt", [N, BL], bf16, kind="ExternalInput")
    w = nc.dram_tensor("w", [D, N], bf16, kind="ExternalInput")
    wt = nc.dram_tensor("wt", [N, D], bf16, kind="ExternalInput")
    bm = nc.dram_tensor("bm", [128, 128], bf16, kind="ExternalInput")
    ot = nc.dram_tensor("ot", [N, BL], fp32, kind="ExternalOutput")

    ms = _mom_coeffs(num_layers)

    with TileContext(nc) as tc:
        with (
            tc.tile_pool(name="wp", bufs=1) as wp,
            tc.tile_pool(name="st", bufs=1) as st,
            tc.tile_pool(name="scr", bufs=2) as scr,
            tc.tile_pool(name="ps_u", bufs=1, space="PSUM") as ps_u,
            tc.tile_pool(name="ps_g", bufs=2, space="PSUM") as ps_g,
            tc.tile_pool(name="ps_s", bufs=2, space="PSUM") as ps_s,
        ):
            bmat = wp.tile([128, 128], bf16, tag="bmat")
            nc.sync.dma_start(out=bmat, in_=bm[:, :])
            eps_t = wp.tile([128, 1], fp32, tag="eps")
            nc.vector.memset(eps_t, EPS)
            wtsb = wp.tile([128, NS, D], bf16, tag="wtsb")
            nc.sync.dma_start(
                out=wtsb, in_=wt.rearrange("(s p) e -> p s e", p=128))
            xts = wp.tile([128, NS, BL], bf16, tag="xts")
            nc.sync.dma_start(
                out=xts, in_=xt.rearrange("(s p) b -> p s b", p=128))
            wsb = wp.tile([128, NT, N], bf16, tag="wsb")
            nc.sync.dma_start(
                out=wsb, in_=w.rearrange("(t p) n -> p t n", p=128))

            # persistent state
            y2 = st.tile([128, FD], fp32, tag="y2")
            xb0 = st.tile([128, FD], fp32, tag="xb0")
            xb1 = st.tile([128, FD], fp32, tag="xb1")
            xbuf = [xb0, xb1]
            uTb = st.tile([128, NS * BL], bf16, tag="uTb")
            # chunked for cross-iteration pipelining (u-phase starts on
            # chunk j as soon as act chunk j lands)
            xsf = [st.tile([128, CHUNK], fp32, tag=f"xsf{j}",
                           name=f"xsf{j}") for j in range(NCH)]
            xtmpb = [st.tile([128, CHUNK], bf16, tag=f"xtmp{j}",
                             name=f"xtmp{j}") for j in range(NCH)]
            pre = [st.tile([128, CHUNK], fp32, tag=f"pre{j}",
                           name=f"pre{j}") for j in range(NCH)]
            vt = [st.tile([128, CHUNK], fp32, tag=f"v{j}",
                          name=f"v{j}") for j in range(NCH)]

            nc.vector.memset(xb0, 0.0)

            # ---- precomp: y2 = TAU * W @ x^T  in [D, b] layout ----
            for t in range(NT):
                py = ps_g.tile([128, BL], fp32, tag="pg")
                for s in range(NS):
                    nc.tensor.matmul(
                        py, wtsb[:, s, t * 128:(t + 1) * 128],
                        xts[:, s, :], start=(s == 0), stop=(s == NS - 1))
                nc.scalar.mul(y2[:, t * BL:(t + 1) * BL], py, TAU)

            def act_block(vch, k):
                """vch(j) -> [128, CHUNK] AP of the pre-activation v.
                Writes xnew + (xsf, xtmpb, pre) for the next iteration; on
                the last iteration writes z (bf16) into xtmpb instead.
                """
                xnew, xold = xbuf[k % 2], xbuf[(k - 1) % 2]
                m = ms[k - 1]
                mr = m / (1.0 + m)
                last = k == num_layers
                for j in range(NCH):
                    sl = slice(j * CHUNK, (j + 1) * CHUNK)
                    vj = vch(j)
                    v2b = scr.tile([128, CHUNK], bf16, tag="v2b")
                    nc.scalar.square(v2b, vj)
                    gs = ps_s.tile([128, CHUNK], fp32, tag="gs")
                    nc.tensor.matmul(gs, bmat, v2b, start=True, stop=True)
                    nrm = scr.tile([128, CHUNK], fp32, tag="nrm")
                    nc.scalar.activation(nrm, gs, AF.Sqrt, bias=eps_t[:, :])
                    invn = scr.tile([128, CHUNK], fp32, tag="invn")
                    nc.vector.reciprocal_approx_fast(invn, nrm)
                    scl = scr.tile([128, CHUNK], fp32, tag="scl")
                    # relu(1 - CTH / nrm)
                    nc.scalar.activation(scl, invn, AF.Relu,
                                         bias=1.0, scale=-CTH)
                    if last:
                        # z (bf16) straight into the xtmp tiles for decode
                        nc.vector.scalar_tensor_tensor(
                            xtmpb[j], vj, 0.0, scl, op0=OP.max, op1=OP.mult)
                        continue
                    # xnew = max(v, 0) * scl
                    nc.vector.scalar_tensor_tensor(
                        xnew[:, sl], vj, 0.0, scl, op0=OP.max, op1=OP.mult)
                    # xs = xnew - m/(1+m) * xold  (fp32 momentum state)
                    nc.vector.scalar_tensor_tensor(
                        xsf[j], xold[:, sl], -mr, xnew[:, sl],
                        op0=OP.mult, op1=OP.add)
                    nc.vector.tensor_copy(xtmpb[j], xsf[j])
                    # pre = (1+m) * xs + y2  (fp32 linear term of v)
                    nc.vector.scalar_tensor_tensor(
                        pre[j], xsf[j], 1.0 + m, y2[:, sl],
                        op0=OP.mult, op1=OP.add)

            # ---- iteration 1: x_tmp = 0 -> v = y2 ----
            act_block(lambda j: y2[:, j * CHUNK:(j + 1) * CHUNK], 1)

            # ---- iterations 2..num_layers ----
            for k in range(2, num_layers + 1):
                # u-phase: uT[n,b] = (1+m_prev) * sum_d W[d,n] xs[d,b]
                # t-outer with 8 concurrent PSUM groups (packed 2 per bank)
                # so the PE consumes act chunks as they land.
                umul = 1.0 + ms[k - 2]
                for half in range(2):
                    pus = [ps_u.tile([128, BL], fp32, tag=f"pu{i}",
                                     name=f"pu{i}") for i in range(NS // 2)]
                    for t in range(NT):
                        for si in range(NS // 2):
                            s = half * (NS // 2) + si
                            nc.tensor.matmul(
                                pus[si],
                                wsb[:, t, s * 128:(s + 1) * 128],
                                xtmpb[t // 2][:, (t % 2) * BL:(t % 2 + 1) * BL],
                                start=(t == 0), stop=(t == NT - 1))
                    for si in range(NS // 2):
                        s = half * (NS // 2) + si
                        dst = uTb[:, s * BL:(s + 1) * BL]
                        if si % 2 == 0:
                            nc.scalar.mul(dst, pus[si], umul)
                        else:
                            nc.vector.tensor_scalar_mul(dst, pus[si], umul)
                # grad-phase + v-combine
                for t in range(NT):
                    pg = ps_g.tile([128, BL], fp32, tag="pg")
                    for s in range(NS):
                        nc.tensor.matmul(
                            pg, wtsb[:, s, t * 128:(t + 1) * 128],
                            uTb[:, s * BL:(s + 1) * BL],
                            start=(s == 0), stop=(s == NS - 1))
                    # v = pre - TAU * grad
                    nc.vector.scalar_tensor_tensor(
                        vt[t // 2][:, (t % 2) * BL:(t % 2 + 1) * BL],
                        pg, -TAU,
                        pre[t // 2][:, (t % 2) * BL:(t % 2 + 1) * BL],
                        op0=OP.mult, op1=OP.add)
                act_block(lambda j: vt[j][:, :], k)

            # ---- decode: out^T[n,b] = sum_d W[d,n] z[d,b] ----
            otsb = st.tile([128, NS, BL], fp32, tag="otsb")
            for half in range(2):
                pds = [ps_u.tile([128, BL], fp32, tag=f"pu{i}",
                                 name=f"pd{i}") for i in range(NS // 2)]
                for t in range(NT):
                    for si in range(NS // 2):
                        s = half * (NS // 2) + si
                        nc.tensor.matmul(
                            pds[si],
                            wsb[:, t, s * 128:(s + 1) * 128],
                            xtmpb[t // 2][:, (t % 2) * BL:(t % 2 + 1) * BL],
                            start=(t == 0), stop=(t == NT - 1))
                for si in range(NS // 2):
                    s = half * (NS // 2) + si
                    if si % 2 == 0:
                        nc.scalar.copy(otsb[:, s, :], pds[si])
                    else:
                        nc.vector.tensor_copy(otsb[:, s, :], pds[si])
            nc.sync.dma_start(
                out=ot.rearrange("(s p) b -> p s b", p=128), in_=otsb)

    nc.compile()
    return nc


_CACHED = {}


def _get_nc(num_layers=NUM_LAYERS):
    if num_layers not in _CACHED:
        _CACHED[num_layers] = build(num_layers)
    return _CACHED[num_layers]


def make_in_maps(x, w):
    """x [B,C,N] fp32, w [C,D,N] fp32 -> list of 8 per-core input dicts."""
    x = np.asarray(x, dtype=np.float32)
    w = np.asarray(w, dtype=np.float32)
    bm = _bmat_np()
    wc = [np.ascontiguousarray(w[c]).astype(ml_dtypes.bfloat16)
          for c in range(C)]
    wtc = [np.ascontiguousarray(w[c].T).astype(ml_dtypes.bfloat16)
           for c in range(C)]
    maps = []
    for i in range(N_CORES):
        c, h = CORE_CH[i], CORE_HALF[i]
        xs = x[h * BL:(h + 1) * BL, c, :]  # [BL, N]
        xts = np.ascontiguousarray(xs.T).astype(ml_dtypes.bfloat16)  # [N, BL]
        maps.append({"xt": xts, "w": wc[c], "wt": wtc[c], "bm": bm})
    return maps


def assemble_out(results):
    out = np.empty((B, C, N), dtype=np.float32)
    for i in REAL_CORES:
        c, h = CORE_CH[i], CORE_HALF[i]
        o = np.asarray(results[i]["ot"], dtype=np.float32)  # [N, BL]
        out[h * BL:(h + 1) * BL, c, :] = o.T
    return out


def kernel(x, W):
    from concourse.bass_utils import run_bass_kernel_spmd

    nc = _get_nc()
    res = run_bass_kernel_spmd(nc, make_in_maps(x, W), list(range(N_CORES)))
    return assemble_out(res.results)


if __name__ == "__main__":
    xs = np.random.randn(B, C, N).astype(np.float32)
    ws = np.random.randn(C, D, N).astype(np.float32)
    ws /= np.linalg.norm(ws, axis=-1, keepdims=True)
    out = kernel(xs, ws)
    print("out", out.shape, out.dtype, float(np.abs(out).mean()))
